# revision 6
# baseline (speedup 1.0000x reference)
"""Single-head self-attention (B=4, S=2048, D=1024) on 8 trn2 NeuronCores.

Sharding: core c -> (batch b = c//2, query half h = c%2). Each core gets a
permuted x^T for its batch (own seq-half first; softmax is invariant to key
permutation), computes Q for its 1024 queries and K/V for all 2048 keys,
then attention. Output rows are the core's own queries in original order, so
the host gather is a pure concatenation.

Device layout (per core):
  xt (input)  : [1024_d, 2048_t] fp32   (x_perm.T, host-prepared)
  Q^T         : [128_dp, 8_dc, 512_s]   per s-block, SBUF
  K^T         : [128_jp, 8_jc, 2048_t]  spilled to HBM scratch, streamed back
  V           : [128_tp, 16_tc, 1024_j] SBUF-resident
  scores^T    : [128_t, 512_s] PSUM -> exp -> SBUF (keys on partitions, so
                attn@V needs no transposes; softmax sum over keys is an
                extra N=1 matmul vs a ones vector sharing the stationary
                operand; max-subtraction skipped: scores ~ N(0, 0.33))
All matmuls fp32r (fp22 mantissa, full PE rate at N>=256, fp32 accumulate).
"""

import os
import sys
import types

import numpy as np

B, S, D = 4, 2048, 1024
HALF = S // 2  # 1024 queries per core
SCALE = 1.0 / 32.0  # 1/sqrt(D)
NC = 8
DC = D // 128  # 8 d-chunks
TT = S // 128  # 16 key tiles
SBLK = 512  # queries per s-block
NSB = HALF // SBLK  # 2 s-blocks

_CACHED_NC = None
LAST_RESULT = None  # BassKernelResults of the most recent run (for test.py)


def _ensure_axon_ntff_hook():
    """bass_utils' trace path needs antenv.axon_hooks; this image's antenv
    lacks it. Install a shim backed by trn_agent_boot's ctypes hook so
    BASS_TRACE=1 profiling works. No-op if already present/unavailable."""
    try:
        import antenv.axon_hooks  # noqa: F401

        return
    except ImportError:
        pass
    try:
        from trn_agent_boot.trn_boot import _ntff_profile_via_ctypes

        hook = _ntff_profile_via_ctypes("/opt/axon/libaxon_pjrt.so")
    except Exception:
        hook = None
    mod = types.ModuleType("antenv.axon_hooks")
    mod.get_axon_ntff_profile_hook = lambda: hook
    mod.set_axon_ntff_profile_hook = lambda h: None
    sys.modules["antenv.axon_hooks"] = mod


def build_kernel(tc, xt, wq, wk, wv, bq, bk, bv, out):
    import concourse.tile as tile  # noqa: F401
    from concourse import mybir

    nc = tc.nc
    F32 = mybir.dt.float32
    F32R = mybir.dt.float32r
    Identity = mybir.ActivationFunctionType.Identity
    Exp = mybir.ActivationFunctionType.Exp

    xt_r = xt.rearrange("(c p) t -> p c t", p=128)  # [128, 8, 2048]
    out_r = out.rearrange("(su p) j -> su p j", p=128)  # [8, 128, 1024]

    with (
        tc.tile_pool(name="persist", bufs=1) as persist,
        tc.tile_pool(name="dram", bufs=1, space="DRAM") as dram,
    ):
        xT_own = persist.tile([128, DC, HALF], F32R)
        nc.sync.dma_start(xT_own, xt_r[:, :, 0:HALF])
        V_sb = persist.tile([128, TT, D], F32R)
        bv_bc = persist.tile([128, D], F32)
        import concourse.bass as bass

        bv_bcast_ap = bass.AP(tensor=bv.tensor, offset=bv.offset, ap=[[0, 128]] + list(bv.ap))
        nc.sync.dma_start(bv_bc, bv_bcast_ap)
        bq_sb = persist.tile([128, DC], F32)
        nc.sync.dma_start(bq_sb, bq)
        bk_sb = persist.tile([128, DC], F32)
        nc.sync.dma_start(bk_sb, bk)
        ones_f = persist.tile([128, 2], F32)
        nc.vector.memset(ones_f, 1.0)
        ones_t = persist.tile([128, 2], F32R)
        nc.vector.tensor_copy(ones_t, ones_f)

        kT_dram = dram.tile([128, DC, S], F32R)

        # ---------------- Phase A: K^T (-> HBM) and V (resident) ----------
        with (
            tc.tile_pool(name="pa", bufs=1) as pa,
            tc.tile_pool(name="pa_w", bufs=2) as paw,
            tc.tile_pool(name="pa_st", bufs=3) as past,
            tc.tile_pool(name="psa", bufs=2, space="PSUM") as psa,
        ):
            xT_oth = pa.tile([128, DC, HALF], F32R)
            nc.sync.dma_start(xT_oth, xt_r[:, :, HALF:S])
            wv_sb = pa.tile([128, DC, D], F32R)
            nc.sync.dma_start(wv_sb, wv)

            def xsl(c, lo, hi):
                # x^T columns [lo,hi) never straddle the HALF boundary here
                if hi <= HALF:
                    return xT_own[:, c, lo:hi]
                return xT_oth[:, c, lo - HALF : hi - HALF]

            # K^T[j, t] = sum_d Wk[d, j] * xT[d, t]  (+bk fused on copy-out)
            for jt in range(DC):
                wk_t = paw.tile([128, DC, 128], F32R, tag="wk_t")
                nc.sync.dma_start(wk_t, wk[:, :, jt * 128 : (jt + 1) * 128])
                for tb in range(S // 512):
                    kpsum = psa.tile([128, 512], F32, tag="kpsum")
                    for c in range(DC):
                        nc.tensor.matmul(
                            kpsum,
                            wk_t[:, c, :],
                            xsl(c, tb * 512, tb * 512 + 512),
                            start=(c == 0),
                            stop=(c == DC - 1),
                        )
                    kstage = past.tile([128, 512], F32R, tag="kstage")
                    nc.scalar.activation(
                        kstage, kpsum, Identity, bias=bk_sb[:, jt : jt + 1]
                    )
                    nc.sync.dma_start(
                        kT_dram[:, jt, tb * 512 : (tb + 1) * 512], kstage
                    )

            # V[t, j] = sum_d xT[d, t] * Wv[d, j]  (+bv via DVE add)
            for tt in range(TT):
                for jb in range(2):
                    vpsum = psa.tile([128, 512], F32, tag="vpsum")
                    for c in range(DC):
                        nc.tensor.matmul(
                            vpsum,
                            xsl(c, tt * 128, tt * 128 + 128),
                            wv_sb[:, c, jb * 512 : (jb + 1) * 512],
                            start=(c == 0),
                            stop=(c == DC - 1),
                        )
                    nc.vector.tensor_add(
                        V_sb[:, tt, jb * 512 : (jb + 1) * 512],
                        vpsum,
                        bv_bc[:, jb * 512 : (jb + 1) * 512],
                    )

        # ---------------- Phase B: Q^T, scores^T, softmax, out ------------
        with (
            tc.tile_pool(name="pb_q", bufs=1) as pbq,
            tc.tile_pool(name="pb_w", bufs=2) as pbw,
            tc.tile_pool(name="pb_kt", bufs=2) as pbkt,
            tc.tile_pool(name="pb_p", bufs=1) as pbp,
            tc.tile_pool(name="pb_o", bufs=2) as pbo,
            tc.tile_pool(name="pb_m", bufs=2) as pbm,
            tc.tile_pool(name="psb_q", bufs=2, space="PSUM") as psbq,
            tc.tile_pool(name="psb_s", bufs=2, space="PSUM") as psbs,
            tc.tile_pool(name="psb_o", bufs=1, space="PSUM") as psbo,
            tc.tile_pool(name="psb_l", bufs=1, space="PSUM") as psbl,
        ):
            for sb in range(NSB):
                # Q^T[dq, s] for this s-block (queries = own half of x^T)
                qT = pbq.tile([128, DC, SBLK], F32R, tag="qT")
                for qc in range(DC):
                    wq_t = pbw.tile([128, DC, 128], F32R, tag="wq_t")
                    nc.sync.dma_start(wq_t, wq[:, :, qc * 128 : (qc + 1) * 128])
                    qpsum = psbq.tile([128, SBLK], F32, tag="qpsum")
                    for c in range(DC):
                        nc.tensor.matmul(
                            qpsum,
                            wq_t[:, c, :],
                            xT_own[:, c, sb * SBLK : (sb + 1) * SBLK],
                            start=(c == 0),
                            stop=(c == DC - 1),
                        )
                    nc.scalar.activation(
                        qT[:, qc, :], qpsum, Identity, bias=bq_sb[:, qc : qc + 1]
                    )

                # scores^T[t, s] = sum_j K^T[j, t] Q^T[j, s]; exp fused w/ 1/32
                expP = pbp.tile([128, TT, SBLK], F32R, tag="expP")
                for tt in range(TT):
                    kt_t = pbkt.tile([128, DC, 128], F32R, tag="kt_t")
                    nc.sync.dma_start(kt_t, kT_dram[:, :, tt * 128 : (tt + 1) * 128])
                    spsum = psbs.tile([128, SBLK], F32, tag="spsum")
                    for jc in range(DC):
                        nc.tensor.matmul(
                            spsum,
                            kt_t[:, jc, :],
                            qT[:, jc, :],
                            start=(jc == 0),
                            stop=(jc == DC - 1),
                        )
                    nc.scalar.activation(expP[:, tt, :], spsum, Exp, scale=SCALE)

                # out[s, j] = sum_t expP[t, s] V[t, j];  l[s] = sum_t expP[t, s]
                for su in range(SBLK // 128):
                    opsum = psbo.tile([128, D], F32, tag="opsum")
                    lpsum = psbl.tile([128, 2], F32, tag="lpsum")
                    for tt in range(TT):
                        lhsT = expP[:, tt, su * 128 : (su + 1) * 128]
                        nc.tensor.matmul(
                            opsum[:, 0:512],
                            lhsT,
                            V_sb[:, tt, 0:512],
                            start=(tt == 0),
                            stop=(tt == TT - 1),
                        )
                        nc.tensor.matmul(
                            opsum[:, 512:1024],
                            lhsT,
                            V_sb[:, tt, 512:1024],
                            start=(tt == 0),
                            stop=(tt == TT - 1),
                        )
                        nc.tensor.matmul(
                            lpsum,
                            lhsT,
                            ones_t,
                            start=(tt == 0),
                            stop=(tt == TT - 1),
                        )
                    recip = pbm.tile([128, 1], F32, tag="recip")
                    nc.vector.reciprocal(recip, lpsum[:, 0:1])
                    o_sb = pbo.tile([128, D], F32, tag="o_sb")
                    nc.vector.tensor_scalar_mul(o_sb, in0=opsum, scalar1=recip)
                    nc.sync.dma_start(out_r[sb * (SBLK // 128) + su], o_sb)


def build_nc():
    global _CACHED_NC
    if _CACHED_NC is not None:
        return _CACHED_NC
    import concourse.tile as tile
    from concourse import bacc, mybir

    F32 = mybir.dt.float32
    F32R = mybir.dt.float32r
    nc = bacc.Bacc("TRN2", target_bir_lowering=False, debug=False)
    xt = nc.dram_tensor("xt", [D, S], F32R, kind="ExternalInput").ap()
    wq = nc.dram_tensor("wq", [128, DC, D], F32R, kind="ExternalInput").ap()
    wk = nc.dram_tensor("wk", [128, DC, D], F32R, kind="ExternalInput").ap()
    wv = nc.dram_tensor("wv", [128, DC, D], F32R, kind="ExternalInput").ap()
    bq = nc.dram_tensor("bq", [128, DC], F32, kind="ExternalInput").ap()
    bk = nc.dram_tensor("bk", [128, DC], F32, kind="ExternalInput").ap()
    bv = nc.dram_tensor("bv", [D], F32, kind="ExternalInput").ap()
    out = nc.dram_tensor("out", [HALF, D], F32, kind="ExternalOutput").ap()

    with tile.TileContext(nc) as tc:
        build_kernel(tc, xt, wq, wk, wv, bq, bk, bv, out)
    nc.compile()
    _CACHED_NC = nc
    return nc


def _shard_inputs(x, Wq, bq, Wk, bk, Wv, bv):
    """Host-side prep: per-core permuted x^T + relaid-out weights/biases."""
    wq_r = np.ascontiguousarray(Wq.reshape(DC, 128, D).transpose(1, 0, 2))
    wk_r = np.ascontiguousarray(Wk.reshape(DC, 128, D).transpose(1, 0, 2))
    wv_r = np.ascontiguousarray(Wv.reshape(DC, 128, D).transpose(1, 0, 2))
    bq_r = np.ascontiguousarray(bq.reshape(DC, 128).T)
    bk_r = np.ascontiguousarray(bk.reshape(DC, 128).T)
    bv_c = np.ascontiguousarray(bv)

    in_maps = []
    for c in range(NC):
        b, h = divmod(c, 2)
        xb = x[b]
        if h:
            xb = np.concatenate([xb[HALF:], xb[:HALF]], axis=0)
        xt = np.ascontiguousarray(xb.T)  # [D, S], own queries first
        in_maps.append(
            {
                "xt": xt,
                "wq": wq_r,
                "wk": wk_r,
                "wv": wv_r,
                "bq": bq_r,
                "bk": bk_r,
                "bv": bv_c,
            }
        )
    return in_maps


def kernel(x, Wq, bq, Wk, bk, Wv, bv):
    global LAST_RESULT
    _ensure_axon_ntff_hook()
    from concourse import bass_utils

    x = np.asarray(x, dtype=np.float32)
    args = [np.asarray(a, dtype=np.float32) for a in (Wq, bq, Wk, bk, Wv, bv)]
    nc = build_nc()
    in_maps = _shard_inputs(x, *args)
    res = bass_utils.run_bass_kernel_spmd(nc, in_maps, core_ids=list(range(NC)))
    LAST_RESULT = res
    out = np.empty((B, S, D), dtype=np.float32)
    for c in range(NC):
        b, h = divmod(c, 2)
        out[b, h * HALF : (h + 1) * HALF, :] = res.results[c]["out"]
    return out


if __name__ == "__main__":
    rng = np.random.default_rng(0)
    init = 1.0 / 32.0
    x = rng.standard_normal((B, S, D), dtype=np.float32)
    mk = lambda *s: rng.uniform(-init, init, s).astype(np.float32)
    o = kernel(x, mk(D, D), mk(D), mk(D, D), mk(D), mk(D, D), mk(D))
    print("out", o.shape, o.dtype, float(np.abs(o).max()))


# revision 7
# speedup vs baseline: 1.2020x; 1.2020x over previous
"""Single-head self-attention (B=4, S=2048, D=1024) on 8 trn2 NeuronCores.

Sharding: core c -> (batch b = c//2, query half h = c%2). Each core gets a
permuted x^T for its batch (own seq-half first; softmax is invariant to key
permutation), computes Q for its 1024 queries and K/V for all 2048 keys,
then attention. Output rows are the core's own queries in original order, so
the host gather is a pure concatenation.

Device layout (per core):
  xt (input)  : [1024_d, 2048_t] fp32   (x_perm.T, host-prepared)
  Q^T         : [128_dp, 8_dc, 512_s]   per s-block, SBUF
  K^T         : [128_jp, 8_jc, 2048_t]  spilled to HBM scratch, streamed back
  V           : [128_tp, 16_tc, 1024_j] SBUF-resident
  scores^T    : [128_t, 512_s] PSUM -> exp -> SBUF (keys on partitions, so
                attn@V needs no transposes; softmax sum over keys is an
                extra N=1 matmul vs a ones vector sharing the stationary
                operand; max-subtraction skipped: scores ~ N(0, 0.33))
All matmuls fp32r (fp22 mantissa, full PE rate at N>=256, fp32 accumulate).
"""

import os
import sys
import types

import numpy as np

B, S, D = 4, 2048, 1024
HALF = S // 2  # 1024 queries per core
SCALE = 1.0 / 32.0  # 1/sqrt(D)
NC = 8
DC = D // 128  # 8 d-chunks
TT = S // 128  # 16 key tiles
SBLK = 512  # queries per s-block
NSB = HALF // SBLK  # 2 s-blocks

_CACHED_NC = None
LAST_RESULT = None  # BassKernelResults of the most recent run (for test.py)


def _ensure_axon_ntff_hook():
    """bass_utils' trace path needs antenv.axon_hooks; this image's antenv
    lacks it. Install a shim backed by trn_agent_boot's ctypes hook so
    BASS_TRACE=1 profiling works. No-op if already present/unavailable."""
    try:
        import antenv.axon_hooks  # noqa: F401

        return
    except ImportError:
        pass
    try:
        from trn_agent_boot.trn_boot import _ntff_profile_via_ctypes

        hook = _ntff_profile_via_ctypes("/opt/axon/libaxon_pjrt.so")
    except Exception:
        hook = None
    mod = types.ModuleType("antenv.axon_hooks")
    mod.get_axon_ntff_profile_hook = lambda: hook
    mod.set_axon_ntff_profile_hook = lambda h: None
    sys.modules["antenv.axon_hooks"] = mod


def build_kernel(tc, xt, wq, wk, wv, bq, bk, bv, out):
    import concourse.bass as bass
    from concourse import mybir

    nc = tc.nc
    F32 = mybir.dt.float32
    F32R = mybir.dt.float32r
    Identity = mybir.ActivationFunctionType.Identity
    Exp = mybir.ActivationFunctionType.Exp

    xt_r = xt.rearrange("(c p) t -> p c t", p=128)  # [128, 8, 2048]
    out_r = out.rearrange("(su p) j -> su p j", p=128)  # [8, 128, 1024]

    with (
        tc.tile_pool(name="persist", bufs=1) as persist,
        tc.tile_pool(name="dram", bufs=1, space="DRAM") as dram,
    ):
        V_sb = persist.tile([128, TT, D], F32R)
        qT = persist.tile([128, DC, HALF], F32R)
        bv_bc = persist.tile([128, D], F32)
        bv_bcast_ap = bass.AP(
            tensor=bv.tensor, offset=bv.offset, ap=[[0, 128]] + list(bv.ap)
        )
        nc.sync.dma_start(bv_bc, bv_bcast_ap)
        bq_sb = persist.tile([128, DC], F32)
        nc.sync.dma_start(bq_sb, bq)
        bk_sb = persist.tile([128, DC], F32)
        nc.sync.dma_start(bk_sb, bk)
        ones_f = persist.tile([128, 2], F32)
        nc.vector.memset(ones_f, 1.0)
        ones_t = persist.tile([128, 2], F32R)
        nc.vector.tensor_copy(ones_t, ones_f)

        kT_dram = dram.tile([128, DC, S], F32R)

        # ------------- Phase A: Q^T, K^T (-> HBM), V (resident) -----------
        with (
            tc.tile_pool(name="pa", bufs=1) as pa,
            tc.tile_pool(name="pa_w", bufs=3) as paw,
            tc.tile_pool(name="pa_st", bufs=2) as past,
            tc.tile_pool(name="psa", bufs=2, space="PSUM") as psa,
        ):
            # x^T loaded in per-chunk DMAs so matmuls start as data lands
            xT = pa.tile([128, DC, S], F32R)
            for c in range(DC):
                nc.sync.dma_start(xT[:, c, :], xt_r[:, c, :])

            # Q^T[dq, s] = sum_d Wq[d, dq] xT[d, s]  (+bq fused on copy-out)
            for qc in range(DC):
                wq_t = paw.tile([128, DC, 128], F32R, tag="w_t")
                nc.sync.dma_start(wq_t, wq[:, :, qc * 128 : (qc + 1) * 128])
                for sblk in range(NSB):
                    qpsum = psa.tile([128, SBLK], F32, tag="qpsum")
                    for c in range(DC):
                        nc.tensor.matmul(
                            qpsum,
                            wq_t[:, c, :],
                            xT[:, c, sblk * SBLK : (sblk + 1) * SBLK],
                            start=(c == 0),
                            stop=(c == DC - 1),
                        )
                    nc.scalar.activation(
                        qT[:, qc, sblk * SBLK : (sblk + 1) * SBLK],
                        qpsum,
                        Identity,
                        bias=bq_sb[:, qc : qc + 1],
                    )

            # K^T[j, t] (+bk) -> HBM scratch
            def k_group(jt):
                wk_t = paw.tile([128, DC, 128], F32R, tag="w_t")
                nc.sync.dma_start(wk_t, wk[:, :, jt * 128 : (jt + 1) * 128])
                for tb in range(S // 512):
                    kpsum = psa.tile([128, 512], F32, tag="kpsum")
                    for c in range(DC):
                        nc.tensor.matmul(
                            kpsum,
                            wk_t[:, c, :],
                            xT[:, c, tb * 512 : (tb + 1) * 512],
                            start=(c == 0),
                            stop=(c == DC - 1),
                        )
                    kstage = past.tile([128, 512], F32R, tag="kstage")
                    nc.scalar.activation(
                        kstage, kpsum, Identity, bias=bk_sb[:, jt : jt + 1]
                    )
                    nc.sync.dma_start(
                        kT_dram[:, jt, tb * 512 : (tb + 1) * 512], kstage
                    )

            # V[t, j] (+bv), one j-half at a time (wv streamed)
            def v_half(jb):
                wv_h = pa.tile([128, DC, 512], F32R, tag="wv_h")
                nc.sync.dma_start(wv_h, wv[:, :, jb * 512 : (jb + 1) * 512])
                for tt in range(TT):
                    vpsum = psa.tile([128, 512], F32, tag="vpsum")
                    for c in range(DC):
                        nc.tensor.matmul(
                            vpsum,
                            xT[:, c, tt * 128 : (tt + 1) * 128],
                            wv_h[:, c, :],
                            start=(c == 0),
                            stop=(c == DC - 1),
                        )
                    nc.vector.tensor_add(
                        V_sb[:, tt, jb * 512 : (jb + 1) * 512],
                        vpsum,
                        bv_bc[:, jb * 512 : (jb + 1) * 512],
                    )

            # interleave so wv half-reloads hide under K-tile matmuls
            for jt in range(4):
                k_group(jt)
            v_half(0)
            for jt in range(4, DC):
                k_group(jt)
            v_half(1)

        # ------------- Phase B: scores^T, softmax, out --------------------
        with (
            tc.tile_pool(name="pb_kt", bufs=2) as pbkt,
            tc.tile_pool(name="pb_p", bufs=1) as pbp,
            tc.tile_pool(name="pb_o", bufs=2) as pbo,
            tc.tile_pool(name="pb_m", bufs=2) as pbm,
            tc.tile_pool(name="psb_s", bufs=2, space="PSUM") as psbs,
            tc.tile_pool(name="psb_o", bufs=2, space="PSUM") as psbo,
            tc.tile_pool(name="psb_l", bufs=2, space="PSUM") as psbl,
        ):
            # scores^T + exp for both s-blocks per K^T tile (K^T read once)
            expP = pbp.tile([128, TT, HALF], F32R, tag="expP")
            for tt in range(TT):
                kt_t = pbkt.tile([128, DC, 128], F32R, tag="kt_t")
                nc.sync.dma_start(kt_t, kT_dram[:, :, tt * 128 : (tt + 1) * 128])
                for sb in range(NSB):
                    spsum = psbs.tile([128, SBLK], F32, tag="spsum")
                    for jc in range(DC):
                        nc.tensor.matmul(
                            spsum,
                            kt_t[:, jc, :],
                            qT[:, jc, sb * SBLK : (sb + 1) * SBLK],
                            start=(jc == 0),
                            stop=(jc == DC - 1),
                        )
                    nc.scalar.activation(
                        expP[:, tt, sb * SBLK : (sb + 1) * SBLK],
                        spsum,
                        Exp,
                        scale=SCALE,
                    )

            # out[s, j] = sum_t expP[t, s] V[t, j];  l[s] via ones column
            for sb in range(NSB):
                for su in range(SBLK // 128):
                    s0 = sb * SBLK + su * 128
                    opsum = psbo.tile([128, D], F32, tag="opsum")
                    lpsum = psbl.tile([128, 2], F32, tag="lpsum")
                    for tt in range(TT):
                        lhsT = expP[:, tt, s0 : s0 + 128]
                        nc.tensor.matmul(
                            opsum[:, 0:512],
                            lhsT,
                            V_sb[:, tt, 0:512],
                            start=(tt == 0),
                            stop=(tt == TT - 1),
                        )
                        nc.tensor.matmul(
                            opsum[:, 512:1024],
                            lhsT,
                            V_sb[:, tt, 512:1024],
                            start=(tt == 0),
                            stop=(tt == TT - 1),
                        )
                        nc.tensor.matmul(
                            lpsum,
                            lhsT,
                            ones_t,
                            start=(tt == 0),
                            stop=(tt == TT - 1),
                        )
                    recip = pbm.tile([128, 1], F32, tag="recip")
                    nc.vector.reciprocal(recip, lpsum[:, 0:1])
                    o_sb = pbo.tile([128, D], F32, tag="o_sb")
                    nc.vector.tensor_scalar_mul(o_sb, in0=opsum, scalar1=recip)
                    nc.sync.dma_start(out_r[sb * (SBLK // 128) + su], o_sb)


def build_nc():
    global _CACHED_NC
    if _CACHED_NC is not None:
        return _CACHED_NC
    import concourse.tile as tile
    from concourse import bacc, mybir

    F32 = mybir.dt.float32
    F32R = mybir.dt.float32r
    nc = bacc.Bacc("TRN2", target_bir_lowering=False, debug=False)
    xt = nc.dram_tensor("xt", [D, S], F32R, kind="ExternalInput").ap()
    wq = nc.dram_tensor("wq", [128, DC, D], F32R, kind="ExternalInput").ap()
    wk = nc.dram_tensor("wk", [128, DC, D], F32R, kind="ExternalInput").ap()
    wv = nc.dram_tensor("wv", [128, DC, D], F32R, kind="ExternalInput").ap()
    bq = nc.dram_tensor("bq", [128, DC], F32, kind="ExternalInput").ap()
    bk = nc.dram_tensor("bk", [128, DC], F32, kind="ExternalInput").ap()
    bv = nc.dram_tensor("bv", [D], F32, kind="ExternalInput").ap()
    out = nc.dram_tensor("out", [HALF, D], F32, kind="ExternalOutput").ap()

    with tile.TileContext(nc) as tc:
        build_kernel(tc, xt, wq, wk, wv, bq, bk, bv, out)
    nc.compile()
    _CACHED_NC = nc
    return nc


def _shard_inputs(x, Wq, bq, Wk, bk, Wv, bv):
    """Host-side prep: per-core permuted x^T + relaid-out weights/biases."""
    wq_r = np.ascontiguousarray(Wq.reshape(DC, 128, D).transpose(1, 0, 2))
    wk_r = np.ascontiguousarray(Wk.reshape(DC, 128, D).transpose(1, 0, 2))
    wv_r = np.ascontiguousarray(Wv.reshape(DC, 128, D).transpose(1, 0, 2))
    bq_r = np.ascontiguousarray(bq.reshape(DC, 128).T)
    bk_r = np.ascontiguousarray(bk.reshape(DC, 128).T)
    bv_c = np.ascontiguousarray(bv)

    in_maps = []
    for c in range(NC):
        b, h = divmod(c, 2)
        xb = x[b]
        if h:
            xb = np.concatenate([xb[HALF:], xb[:HALF]], axis=0)
        xt = np.ascontiguousarray(xb.T)  # [D, S], own queries first
        in_maps.append(
            {
                "xt": xt,
                "wq": wq_r,
                "wk": wk_r,
                "wv": wv_r,
                "bq": bq_r,
                "bk": bk_r,
                "bv": bv_c,
            }
        )
    return in_maps


def kernel(x, Wq, bq, Wk, bk, Wv, bv):
    global LAST_RESULT
    _ensure_axon_ntff_hook()
    from concourse import bass_utils

    x = np.asarray(x, dtype=np.float32)
    args = [np.asarray(a, dtype=np.float32) for a in (Wq, bq, Wk, bk, Wv, bv)]
    nc = build_nc()
    in_maps = _shard_inputs(x, *args)
    res = bass_utils.run_bass_kernel_spmd(nc, in_maps, core_ids=list(range(NC)))
    LAST_RESULT = res
    out = np.empty((B, S, D), dtype=np.float32)
    for c in range(NC):
        b, h = divmod(c, 2)
        out[b, h * HALF : (h + 1) * HALF, :] = res.results[c]["out"]
    return out


if __name__ == "__main__":
    rng = np.random.default_rng(0)
    init = 1.0 / 32.0
    x = rng.standard_normal((B, S, D), dtype=np.float32)
    mk = lambda *s: rng.uniform(-init, init, s).astype(np.float32)
    o = kernel(x, mk(D, D), mk(D), mk(D, D), mk(D), mk(D, D), mk(D))
    print("out", o.shape, o.dtype, float(np.abs(o).max()))
